# revision 23
# baseline (speedup 1.0000x reference)
"""Trainium2 Bass kernel for nn_AttentionBlock (N=32, T=1024, C=K=V=512).

Strategy: data-parallel over batch N across 8 NeuronCores (4 batches/core),
no collectives. Per batch on-core:
  xT = transpose(x) via bf16 DMA-XBAR transpose (DRAM round trip), produced
       one batch ahead so the chain hides under the previous batch's compute
  xT8 = fp8(xT)
  qT = 64*(Wq^T x + bq), kT = 64*(Wk^T x + bk) in fp8-E4M3 via DoubleRow
       matmuls (weights pre-scaled by 64 so W entries clear the E4M3
       min-normal 2^-6; the 64*64 factor is divided back out in the exp)
  v = xT^T Wv + bv  (bf16 matmuls; precision-critical path stays bf16)
  scoresT[s,t] = kT qT^T via fp8 DoubleRow (only tiles t >= s; strict
  lower-tri of the diagonal tile masked with a large negative bias)
  attnT = exp(scoresT/(4096*sqrt(K))) with row sums accumulated in the same
  scalar-engine pass (softmax over the query axis t, per reference)
  vs[s,:] = v[s,:] / rowsum[s]
  attn_out[t,:] = sum_s attnT[s,t] vs[s,:]  (bf16, only s-chunks <= t-chunk)
  out = [x, attn_out]

DMA routing: bulk transfers (x loads, weights, outputs) go through gpsimd
SWDGE; the latency-critical xd-write + XBAR-transpose chain owns the SP
HWDGE ring; casts/exp run on the scalar engine.
"""

import contextlib
import math

import numpy as np

import concourse.bass as bass
import concourse.tile as tile
from bass_rust import add_dep_helper
from concourse import bacc, mybir
from concourse.bass_utils import run_bass_kernel_spmd

N, T, C, K, V = 32, 1024, 512, 512, 512
NCORES = 8
NB = N // NCORES  # batches per core
P = 128
CO = C // P  # 4 chunks of contraction dim
KO = K // P  # 4 chunks of qk feature dim
TO = T // P  # 8 chunks of sequence dim
F32 = mybir.dt.float32
BF16 = mybir.dt.bfloat16
F8 = mybir.dt.float8e4
DR = mybir.MatmulPerfMode.DoubleRow
WSCALE = 64.0  # fp8 weight pre-scale; q,k come out scaled by 64 each
SCALE = 1.0 / (math.sqrt(K) * WSCALE * WSCALE)
NEG = -1.0e13  # masked-score bias; NEG*SCALE ~ -1e11 -> exp == 0


def _body(nc, tc, x_ext, w_exts, b_exts, out_ext, reps=1):
    ctxs = []

    def pool(name, bufs, space="SBUF"):
        p = tc.tile_pool(name=name, bufs=bufs, space=space)
        ctxs.append(p)
        return p.__enter__()

    consts = pool("consts", 1)
    wstage = pool("wstage", 1)
    xdram_pool = pool("xdram", 4, space="DRAM")
    xt_pool = pool("xt", 3)
    xt8_pool = pool("xt8", 2)
    qk_pool = pool("qk", 2)
    at_pool = pool("at", 2)
    small = pool("small", 4)
    ob_pool = pool("ob", 4)
    pp = pool("pp", 4, space="PSUM")
    pav = pool("pav", 3, space="PSUM")
    pwarm = pool("pwarm", 1, space="PSUM")
    pools = (
        xdram_pool,
        xt_pool,
        xt8_pool,
        qk_pool,
        at_pool,
        small,
        ob_pool,
        pp,
        pav,
        pwarm,
    )

    # ---- constants ----
    # maskbias[s_local, t_local]: 0 where t >= s, NEG where t < s
    maskbias = consts.tile([P, P], F32)
    nc.gpsimd.memset(maskbias, 0.0)
    nc.gpsimd.affine_select(
        out=maskbias,
        in_=maskbias,
        compare_op=mybir.AluOpType.is_ge,
        fill=NEG,
        base=0,
        pattern=[[1, P]],  # +1 per t (free)
        channel_multiplier=-1,  # -1 per s (partition); keep where t - s >= 0
    )

    def load_w(name, w_ext, dtype, scale, defer_anchor=None):
        stage = wstage.tile([P, CO, 512], F32, tag="wstage", name=f"stage_{name}")
        dma = nc.gpsimd.dma_start(
            out=stage, in_=w_ext.rearrange("(co p) k -> p co k", p=P)
        )
        if defer_anchor is not None:
            add_dep_helper(dma.ins, defer_anchor.ins, reason="defer behind xT chain")
        wt = consts.tile([P, CO, 512], dtype, tag=f"w_{name}", name=f"w_{name}")
        if scale == 1.0:
            nc.vector.tensor_copy(out=wt, in_=stage)
        else:
            nc.vector.tensor_scalar_mul(out=wt, in0=stage, scalar1=scale)
        return wt

    w_ts = [None, None, None]
    bq_t = consts.tile([P, KO], F32, tag="bq")
    bk_t = consts.tile([P, KO], F32, tag="bk")
    b64 = consts.tile([P, 2, KO], F32, tag="b64")
    bv_b = consts.tile([P, V], F32, tag="bv")

    def early_setup():
        w_ts[0] = load_w("q", w_exts[0], F8, WSCALE)
        nc.gpsimd.dma_start(out=bq_t, in_=b_exts[0].rearrange("(ko p) -> p ko", p=P))
        nc.vector.tensor_scalar_mul(out=b64[:, 0], in0=bq_t, scalar1=WSCALE)

    def late_setup(anchor):
        w_ts[1] = load_w("k", w_exts[1], F8, WSCALE, anchor)
        # Wv: direct SWDGE cast f32 -> bf16 during the DMA, no staging
        wv_bf = consts.tile([P, CO, 512], BF16, tag="w_v", name="w_v")
        dma = nc.gpsimd.dma_start(
            out=wv_bf, in_=w_exts[2].rearrange("(co p) k -> p co k", p=P)
        )
        add_dep_helper(dma.ins, anchor.ins, reason="defer behind xT chain")
        w_ts[2] = wv_bf
        dma = nc.gpsimd.dma_start(
            out=bk_t, in_=b_exts[1].rearrange("(ko p) -> p ko", p=P)
        )
        add_dep_helper(dma.ins, anchor.ins, reason="defer behind xT chain")
        nc.vector.tensor_scalar_mul(out=b64[:, 1], in0=bk_t, scalar1=WSCALE)
        bv_src = bass.AP(
            tensor=b_exts[2].tensor,
            offset=b_exts[2].offset,
            ap=[[0, P]] + list(b_exts[2].ap),
        )
        dma = nc.gpsimd.dma_start(out=bv_b, in_=bv_src)
        add_dep_helper(dma.ins, anchor.ins, reason="defer behind xT chain")

    loop = tc.For_i(0, reps, 1) if reps > 1 else contextlib.nullcontext()
    with loop:
        _batches(
            nc,
            tc,
            x_ext,
            out_ext,
            w_ts,
            b64,
            bv_b,
            maskbias,
            pools,
            early_setup,
            late_setup,
        )

    for p in reversed(ctxs):
        p.__exit__(None, None, None)


def _batches(
    nc, tc, x_ext, out_ext, w_ts, b64, bv_b, maskbias, pools, early_setup,
    late_setup,
):
    (
        xdram_pool,
        xt_pool,
        xt8_pool,
        qk_pool,
        at_pool,
        small,
        ob_pool,
        pp,
        pav,
        pwarm,
    ) = pools

    def xT_stage(n, prev_last_tr):
        """x --(DRAM->DRAM cast to bf16)--> xd --(XBAR transpose)--> xT.

        The D2D cast reads x_ext directly, so the chain has no SBUF WAR
        deps and batch n's chain can start arbitrarily early."""
        xd = xdram_pool.tile([T, C], BF16, tag="xd", name=f"xd_{n}")
        half = T // 2
        for h in range(2):
            sl = slice(h * half, (h + 1) * half)
            d2d = nc.gpsimd.dma_start(out=xd[sl, :], in_=x_ext[n, sl, :])
            if prev_last_tr is not None:
                add_dep_helper(
                    d2d.ins,
                    prev_last_tr.ins,
                    reason="defer prefetch behind xT chain",
                )
        xT = xt_pool.tile([P, CO, T], BF16, tag="xT", name=f"xT_{n}")
        trs = [
            nc.sync.dma_start_transpose(xT[:, co, :], xd[:, P * co : P * (co + 1)])
            for co in range(CO)
        ]
        return None, xT, trs[-1]

    staged = xT_stage(0, None)
    if early_setup is not None:
        early_setup()
        scratch = small.tile([P, 512], F32, tag="warm_rhs", name="warm_rhs")
        nc.vector.memset(scratch, 0.0)
        wpsum = pwarm.tile([P, 512], F32, tag="warm_ps", name="warm_ps")
        nbig, nsmall = 9, 8
        for d in range(nbig):
            nc.tensor.matmul(
                wpsum, lhsT=maskbias, rhs=scratch, start=(d == 0), stop=False
            )
        for d in range(nsmall):
            nc.tensor.matmul(
                wpsum[:, 0:128],
                lhsT=maskbias,
                rhs=scratch[:, 0:128],
                start=False,
                stop=(d == nsmall - 1),
            )
    state = {}

    def stage_proj(n, chain):
        """xT8 cast + q/k fp8-DR projections + v bf16 projection for batch n."""
        _, xT, last_tr = chain
        # fp8 copy of xT for the q/k/scores path (split so q/k matmuls on the
        # first co-pair can start while the second pair is still casting)
        xT8 = xt8_pool.tile([P, CO, T], F8, tag="xT8", name=f"xT8_{n}")
        nc.scalar.copy(out=xT8[:, 0:2, :], in_=xT[:, 0:2, :])
        nc.scalar.copy(out=xT8[:, 2:4, :], in_=xT[:, 2:4, :])

        qT = qk_pool.tile([P, KO, T], F8, tag="qT", name=f"qT_{n}")
        kT = qk_pool.tile([P, KO, T], F8, tag="kT", name=f"kT_{n}")
        for wi, dst, wname in ((0, qT, "q"), (1, kT, "k")):
            wt = w_ts[wi]
            for ko in range(KO):
                pss = [
                    pp.tile([P, 512], F32, tag="psA", name=f"psp_{n}_{wname}_{ko}_{th}")
                    for th in range(2)
                ]
                for j in range(2):
                    for th in range(2):
                        mm = nc.tensor.matmul(
                            pss[th],
                            lhsT=wt[:, 2 * j : 2 * j + 2, P * ko : P * (ko + 1)],
                            rhs=xT8[:, 2 * j : 2 * j + 2, 512 * th : 512 * (th + 1)],
                            start=(j == 0),
                            stop=(j == 1),
                            perf_mode=DR,
                        )
                        if n == 0 and ko == 0 and th == 0 and j == 0 and dst is qT:
                            add_dep_helper(
                                mm.ins,
                                last_tr.ins,
                                reason="start PE only when xT complete",
                            )
                for th in range(2):
                    dst_ap = dst[:, ko, 512 * th : 512 * (th + 1)]
                    if wi == 0:  # q casts on ACT, k casts on DVE (balance)
                        nc.scalar.activation(
                            out=dst_ap,
                            in_=pss[th],
                            func=mybir.ActivationFunctionType.Identity,
                            bias=b64[:, wi, ko : ko + 1],
                            scale=1.0,
                        )
                    else:
                        nc.vector.tensor_scalar_add(
                            out=dst_ap,
                            in0=pss[th],
                            scalar1=b64[:, wi, ko : ko + 1],
                        )
        v_bf = qk_pool.tile([P, TO, V], BF16, tag="v", name=f"v_{n}")
        for so in range(TO):
            ps = pp.tile([P, 512], F32, tag="psA", name=f"psv_{n}_{so}")
            for ci in range(CO):
                nc.tensor.matmul(
                    ps,
                    lhsT=xT[:, ci, P * so : P * (so + 1)],
                    rhs=w_ts[2][:, ci, :],
                    start=(ci == 0),
                    stop=(ci == CO - 1),
                )
            nc.vector.tensor_tensor(
                out=v_bf[:, so, :], in0=ps, in1=bv_b, op=mybir.AluOpType.add
            )
        state[n] = dict(qT=qT, kT=kT, v_bf=v_bf)

    def stage_scores(n):
        """scores fp8-DR matmuls + masked softmax over t + vs for batch n."""
        st = state[n]
        qT, kT, v_bf = st["qT"], st["kT"], st["v_bf"]
        attnT = at_pool.tile([P, TO, T], BF16, tag="attnT", name=f"attnT_{n}")
        vs = qk_pool.tile([P, TO, V], BF16, tag="vs", name=f"vs_{n}")
        recips = small.tile([P, TO], F32, tag="recips", name=f"recips_{n}")
        first_exp = None
        for i in range(TO):
            segs = []
            for th in range(2):
                seg_lo = max(512 * th, P * i)
                seg_hi = 512 * (th + 1)
                if seg_hi > seg_lo:
                    segs.append((th, seg_lo, seg_hi))
            ps_map = {
                th: pp.tile([P, 512], F32, tag="psA", name=f"pss_{n}_{i}_{th}")[
                    :, : hi - lo
                ]
                for th, lo, hi in segs
            }
            for j in range(2):
                for th, lo, hi in segs:
                    nc.tensor.matmul(
                        ps_map[th],
                        lhsT=kT[:, 2 * j : 2 * j + 2, P * i : P * (i + 1)],
                        rhs=qT[:, 2 * j : 2 * j + 2, lo:hi],
                        start=(j == 0),
                        stop=(j == 1),
                        perf_mode=DR,
                    )
            parts = []
            for th, seg_lo, seg_hi in segs:
                ps = ps_map[th]
                if seg_lo == P * i:  # segment starts at the diagonal block
                    nc.vector.tensor_tensor(
                        out=ps[:, 0:P],
                        in0=ps[:, 0:P],
                        in1=maskbias,
                        op=mybir.AluOpType.add,
                    )
                acc = small.tile([P, 1], F32, tag="acc", name=f"acc_{n}_{i}_{th}")
                exp_inst = nc.scalar.activation(
                    out=attnT[:, i, seg_lo:seg_hi],
                    in_=ps,
                    func=mybir.ActivationFunctionType.Exp,
                    scale=SCALE,
                    accum_out=acc,
                )
                if first_exp is None:
                    first_exp = exp_inst
                parts.append(acc)
            if len(parts) == 2:
                rsum = small.tile([P, 1], F32, tag="rsum", name=f"rsum_{n}_{i}")
                nc.vector.tensor_add(out=rsum, in0=parts[0], in1=parts[1])
            else:
                rsum = parts[0]
            nc.vector.reciprocal(out=recips[:, i : i + 1], in_=rsum)
            nc.vector.tensor_scalar_mul(
                out=vs[:, i, :], in0=v_bf[:, i, :], scalar1=recips[:, i : i + 1]
            )

        xcopy = nc.gpsimd.dma_start(out=out_ext[n, :, 0:C], in_=x_ext[n])
        if n < NB - 1:
            add_dep_helper(
                xcopy.ins, first_exp.ins, reason="defer x-copy behind scores"
            )
        st["attnT"] = attnT
        st["vs"] = vs

    def stage_av(n):
        """attn @ v + output writes for batch n."""
        st = state[n]
        attnT, vs = st["attnT"], st["vs"]
        o_view = out_ext[n, :, C : C + V].rearrange("(to p) c -> p to c", p=P)
        j_order = range(TO - 1, -1, -1) if n == NB - 1 else range(TO)
        for j in j_order:
            ps = pav.tile([P, 512], F32, tag="psav", name=f"psav_{n}_{j}")
            for i in range(j + 1):
                nc.tensor.matmul(
                    ps,
                    lhsT=attnT[:, i, P * j : P * (j + 1)],
                    rhs=vs[:, i, :],
                    start=(i == 0),
                    stop=(i == j),
                )
            o_j = ob_pool.tile([P, V], F32, tag="o", name=f"o_{n}_{j}")
            nc.scalar.copy(out=o_j, in_=ps)
            nc.gpsimd.dma_start(out=o_view[:, j : j + 1, :], in_=o_j)
        del state[n]

    # Two-deep software pipeline: emit proj(n+1) ahead of av(n) so PE has
    # projection matmuls queued while batch n's softmax tail completes.
    chains = {0: staged}
    if late_setup is not None:
        late_setup(staged[2])
    chains[1] = xT_stage(1, staged[2])
    chains[2] = xT_stage(2, None)
    stage_proj(0, chains[0])
    stage_scores(0)
    for n in range(1, NB):
        if n + 2 < NB:
            chains[n + 2] = xT_stage(n + 2, None)
        stage_proj(n, chains[n])
        stage_av(n - 1)
        stage_scores(n)
    stage_av(NB - 1)


def build_nc(reps=1):
    nc = bacc.Bacc("TRN2", target_bir_lowering=False, debug=False, num_devices=NCORES)
    x_ext = nc.dram_tensor("x", [NB, T, C], F32, kind="ExternalInput").ap()
    wq = nc.dram_tensor("Wq", [C, K], F32, kind="ExternalInput").ap()
    bq = nc.dram_tensor("bq", [K], F32, kind="ExternalInput").ap()
    wk = nc.dram_tensor("Wk", [C, K], F32, kind="ExternalInput").ap()
    bk = nc.dram_tensor("bk", [K], F32, kind="ExternalInput").ap()
    wv = nc.dram_tensor("Wv", [C, V], F32, kind="ExternalInput").ap()
    bv = nc.dram_tensor("bv", [V], F32, kind="ExternalInput").ap()
    out_ext = nc.dram_tensor("out", [NB, T, C + V], F32, kind="ExternalOutput").ap()

    with tile.TileContext(nc) as tc:
        _body(nc, tc, x_ext, (wq, wk, wv), (bq, bk, bv), out_ext, reps=reps)
    nc.compile()
    return nc


def make_in_maps(x, Wq, bq, Wk, bk, Wv, bv):
    x = np.ascontiguousarray(np.asarray(x, dtype=np.float32))
    return [
        {
            "x": x[NB * i : NB * (i + 1)],
            "Wq": np.asarray(Wq, np.float32),
            "bq": np.asarray(bq, np.float32),
            "Wk": np.asarray(Wk, np.float32),
            "bk": np.asarray(bk, np.float32),
            "Wv": np.asarray(Wv, np.float32),
            "bv": np.asarray(bv, np.float32),
        }
        for i in range(NCORES)
    ]


def kernel(x, Wq, bq, Wk, bk, Wv, bv):
    nc = build_nc()
    in_maps = make_in_maps(x, Wq, bq, Wk, bk, Wv, bv)
    res = run_bass_kernel_spmd(nc, in_maps, list(range(NCORES)))
    return np.concatenate([res.results[i]["out"] for i in range(NCORES)], axis=0)
